# revision 8
# baseline (speedup 1.0000x reference)
"""Trainium2 Bass kernel for the ReLU bound-relaxation (CROWN-style) module.

Full inputs:  l, u: [16, 4096, 1025] f32;  in_lb, in_ub: [1024] f32
Full outputs: (l_new, u_new, post_conc_lb, post_conc_ub)

Math reformulation (validated to ~1e-7 rel err vs the jax reference):
  c = (in_lb+in_ub)/2, r = (in_ub-in_lb)/2  (r >= 0)
  S   = b + sum_e W[n,e]*c[e]          (W = eq coeffs, b = eq bias)
  t   = sum_e |W[n,e]*r[e]|            (== |W|.r since r >= 0)
  conc_lb = S_l - t_l ; max_lb = S_l + t_l ; min_ub = S_u - t_u ; conc_ub = S_u + t_u
  The ReLU relaxation is a per-row scaling:
    l_new = coef_l * l ;  u_new = coef_u * u (+ d_u added to the bias column)
  and therefore the post-concretize pass is analytic:
    post_conc_lb = coef_l * conc_lb ;  post_conc_ub = coef_u * conc_ub + d_u

Sharding: batch axis B=16 split across 8 cores (2 batches/core); no
communication needed.  Per core: 8192 rows of 1025 f32.

Per-core kernel structure (memory-bound target):
  - 64 row-tiles of [128, 1025].  DVE computes S via one fused
    tensor_tensor_reduce (bias as the reduce initial value) and W*r via one
    tensor_tensor; ACT computes t via activation(Abs, accum_out) in-place.
  - dots staged in [128, G] arrays (G=16 tiles/group); branchless coef math
    (~29 small DVE ops per group) amortized across the group.
  - ACT applies coef_l / coef_u as per-partition activation scales in-place,
    then tiles are DMAed out.
"""

import numpy as np

B, N, E = 16, 4096, 1025
NCORES = 8
BPC = B // NCORES          # batches per core
ROWS = BPC * N             # rows per core
P = 128                    # partitions (rows per tile)
NT = ROWS // P             # row-tiles per core
G = 16                     # tiles per coef group
NG = NT // G

_cache = {}


def _build():
    from concourse import bacc, tile, mybir

    f32 = mybir.dt.float32
    Alu = mybir.AluOpType
    Act = mybir.ActivationFunctionType

    nc = bacc.Bacc("TRN2", target_bir_lowering=False, debug=False)

    l_ext = nc.dram_tensor("l", [ROWS, E], f32, kind="ExternalInput").ap()
    u_ext = nc.dram_tensor("u", [ROWS, E], f32, kind="ExternalInput").ap()
    # cbx = [c, 1.0]: sum(W_ext * cbx) over all E cols = b + sum(W*c)
    # rbx = [r, 0.0]: sum|W_ext * rbx| = sum|W*r| = |W|.r  (r >= 0)
    cb_ext = nc.dram_tensor("cbx", [P, E], f32, kind="ExternalInput").ap()
    rb_ext = nc.dram_tensor("rbx", [P, E], f32, kind="ExternalInput").ap()
    lnew_ext = nc.dram_tensor("l_new", [ROWS, E], f32, kind="ExternalOutput").ap()
    unew_ext = nc.dram_tensor("u_new", [ROWS, E], f32, kind="ExternalOutput").ap()
    plb_ext = nc.dram_tensor("post_lb", [NT, P], f32, kind="ExternalOutput").ap()
    pub_ext = nc.dram_tensor("post_ub", [NT, P], f32, kind="ExternalOutput").ap()

    with tile.TileContext(nc) as tc:
        from contextlib import ExitStack

        with ExitStack() as ctx:
            consts = ctx.enter_context(tc.tile_pool(name="consts", bufs=1))
            wpool = ctx.enter_context(tc.tile_pool(name="w", bufs=G + 1))
            prodp = ctx.enter_context(tc.tile_pool(name="prod", bufs=3))
            sinkp = ctx.enter_context(tc.tile_pool(name="sink", bufs=1))
            stage = ctx.enter_context(tc.tile_pool(name="stage", bufs=2))

            cb_t = consts.tile([P, E], f32, tag="cb")
            nc.sync.dma_start(cb_t[:], cb_ext[:, :])
            rb_t = consts.tile([P, E], f32, tag="rb")
            nc.sync.dma_start(rb_t[:], rb_ext[:, :])
            sink = sinkp.tile([P, E], f32, tag="sink")  # stt mandatory out

            for grp in range(NG):
                st = lambda tag: stage.tile([P, G], f32, name=tag, tag=tag)
                Sl, Tl, Su, Tu = st("Sl"), st("Tl"), st("Su"), st("Tu")

                lts, uts = [], []
                for gi in range(G):
                    t = grp * G + gi
                    rows = slice(t * P, (t + 1) * P)
                    lt = wpool.tile([P, E], f32, tag="lt")
                    nc.sync.dma_start(lt[:], l_ext[rows, :])
                    ut = wpool.tile([P, E], f32, tag="ut")
                    nc.sync.dma_start(ut[:], u_ext[rows, :])
                    prl = prodp.tile([P, E], f32, tag="prl")
                    pru = prodp.tile([P, E], f32, tag="pru")

                    # S_l = b_l + sum(W_l * c) : fused multiply+accumulate
                    nc.vector.scalar_tensor_tensor(
                        sink[:], lt[:], 0.0, cb_t[:],
                        op0=Alu.bypass, op1=Alu.mult,
                        accum_out=Sl[:, gi : gi + 1],
                    )
                    # prl = W_l * r ; t_l = sum |prl|  (ACT Abs with accumulate)
                    nc.vector.tensor_mul(prl[:], lt[:], rb_t[:])
                    nc.scalar.activation(
                        prl[:], prl[:], Act.Abs, accum_out=Tl[:, gi : gi + 1]
                    )
                    nc.vector.scalar_tensor_tensor(
                        sink[:], ut[:], 0.0, cb_t[:],
                        op0=Alu.bypass, op1=Alu.mult,
                        accum_out=Su[:, gi : gi + 1],
                    )
                    nc.vector.tensor_mul(pru[:], ut[:], rb_t[:])
                    nc.scalar.activation(
                        pru[:], pru[:], Act.Abs, accum_out=Tu[:, gi : gi + 1]
                    )
                    lts.append(lt)
                    uts.append(ut)

                # ---- branchless coef math on [P, G] ----
                v = nc.vector
                conc_lb, max_lb, conc_ub, min_ub = (
                    st("conc_lb"), st("max_lb"), st("conc_ub"), st("min_ub"))
                v.tensor_sub(conc_lb[:], Sl[:], Tl[:])
                v.tensor_add(max_lb[:], Sl[:], Tl[:])
                v.tensor_add(conc_ub[:], Su[:], Tu[:])
                v.tensor_sub(min_ub[:], Su[:], Tu[:])

                m_ub, m_lb, sum_lu, m_ma = st("m_ub"), st("m_lb"), st("sum_lu"), st("m_ma")
                v.tensor_scalar(m_ub[:], conc_ub[:], 0.0, None, op0=Alu.is_gt)
                v.tensor_scalar(m_lb[:], conc_lb[:], 0.0, None, op0=Alu.is_lt)
                v.tensor_add(sum_lu[:], conc_lb[:], conc_ub[:])
                v.tensor_scalar(m_ma[:], sum_lu[:], 0.0, None, op0=Alu.is_ge)

                tlc, inv_l, q, a_l = st("tlc"), st("inv_l"), st("q"), st("a_l")
                v.tensor_scalar_max(tlc[:], Tl[:], 1e-35)
                v.reciprocal(inv_l[:], tlc[:])
                v.scalar_tensor_tensor(
                    q[:], max_lb[:], 0.5, inv_l[:], op0=Alu.mult, op1=Alu.mult)
                v.tensor_scalar_max(a_l[:], q[:], 0.0)

                cu0, x, y, coef_l, post_l = st("cu0"), st("x"), st("y"), st("coef_l"), st("post_l")
                v.tensor_mul(cu0[:], m_ma[:], a_l[:])
                v.tensor_mul(x[:], m_lb[:], cu0[:])
                v.scalar_tensor_tensor(
                    y[:], x[:], 1.0, m_lb[:], op0=Alu.add, op1=Alu.subtract)
                v.tensor_mul(coef_l[:], m_ub[:], y[:])
                v.tensor_mul(post_l[:], coef_l[:], conc_lb[:])

                tuc, inv_u, a_u = st("tuc"), st("inv_u"), st("a_u")
                v.tensor_scalar_max(tuc[:], Tu[:], 1e-35)
                v.reciprocal(inv_u[:], tuc[:])
                v.scalar_tensor_tensor(
                    a_u[:], conc_ub[:], 0.5, inv_u[:], op0=Alu.mult, op1=Alu.mult)

                m_zc, unst, zc, xu, yu = st("m_zc"), st("unst"), st("zc"), st("xu"), st("yu")
                v.tensor_scalar(m_zc[:], min_ub[:], 0.0, None, op0=Alu.is_le)
                v.tensor_mul(unst[:], m_lb[:], m_ub[:])
                v.tensor_mul(zc[:], unst[:], m_zc[:])
                v.tensor_mul(xu[:], zc[:], a_u[:])
                v.scalar_tensor_tensor(
                    yu[:], xu[:], 1.0, zc[:], op0=Alu.add, op1=Alu.subtract)

                coef_u, du, pu0, post_u = st("coef_u"), st("du"), st("pu0"), st("post_u")
                v.tensor_mul(coef_u[:], m_ub[:], yu[:])
                v.scalar_tensor_tensor(
                    du[:], min_ub[:], -1.0, xu[:], op0=Alu.mult, op1=Alu.mult)
                v.tensor_mul(pu0[:], coef_u[:], conc_ub[:])
                v.tensor_add(post_u[:], pu0[:], du[:])

                # ---- apply scales in-place and DMA out ----
                for gi in range(G):
                    t = grp * G + gi
                    rows = slice(t * P, (t + 1) * P)
                    lt, ut = lts[gi], uts[gi]
                    nc.scalar.activation(
                        lt[:], lt[:], Act.Copy, bias=0.0,
                        scale=coef_l[:, gi : gi + 1])
                    nc.sync.dma_start(lnew_ext[rows, :], lt[:])
                    nc.scalar.activation(
                        ut[:, 0 : E - 1], ut[:, 0 : E - 1], Act.Copy, bias=0.0,
                        scale=coef_u[:, gi : gi + 1])
                    nc.scalar.activation(
                        ut[:, E - 1 : E], ut[:, E - 1 : E], Act.Identity,
                        bias=du[:, gi : gi + 1], scale=coef_u[:, gi : gi + 1])
                    nc.sync.dma_start(unew_ext[rows, :], ut[:])
                    nc.sync.dma_start(plb_ext[t, :], post_l[:, gi : gi + 1])
                    nc.sync.dma_start(pub_ext[t, :], post_u[:, gi : gi + 1])

    nc.compile()
    return nc


def _get_nc():
    if "nc" not in _cache:
        _cache["nc"] = _build()
    return _cache["nc"]


def _run(l, u, in_lb, in_ub, trace=False, runner=None):
    from concourse.bass_utils import run_bass_kernel_spmd

    nc = _get_nc()

    l = np.ascontiguousarray(l, dtype=np.float32)
    u = np.ascontiguousarray(u, dtype=np.float32)
    in_lb = np.asarray(in_lb, dtype=np.float32)
    in_ub = np.asarray(in_ub, dtype=np.float32)

    c = (in_lb + in_ub) * np.float32(0.5)
    r = (in_ub - in_lb) * np.float32(0.5)
    cbx = np.concatenate([c, np.ones(1, np.float32)])
    rbx = np.concatenate([r, np.zeros(1, np.float32)])
    cb = np.ascontiguousarray(np.broadcast_to(cbx, (P, E)), dtype=np.float32)
    rb = np.ascontiguousarray(np.broadcast_to(rbx, (P, E)), dtype=np.float32)

    in_maps = []
    for i in range(NCORES):
        sl = slice(i * BPC, (i + 1) * BPC)
        in_maps.append({
            "l": l[sl].reshape(ROWS, E),
            "u": u[sl].reshape(ROWS, E),
            "cbx": cb,
            "rbx": rb,
        })

    if runner is not None:
        res = runner(nc, in_maps)
    else:
        res = run_bass_kernel_spmd(nc, in_maps, core_ids=list(range(NCORES)),
                                   trace=trace)

    l_new = np.empty((B, N, E), dtype=np.float32)
    u_new = np.empty((B, N, E), dtype=np.float32)
    post_lb = np.empty((B, N), dtype=np.float32)
    post_ub = np.empty((B, N), dtype=np.float32)
    for i in range(NCORES):
        sl = slice(i * BPC, (i + 1) * BPC)
        out = res.results[i]
        l_new[sl] = np.asarray(out["l_new"]).reshape(BPC, N, E)
        u_new[sl] = np.asarray(out["u_new"]).reshape(BPC, N, E)
        post_lb[sl] = np.asarray(out["post_lb"]).reshape(BPC, N)
        post_ub[sl] = np.asarray(out["post_ub"]).reshape(BPC, N)

    return (l_new, u_new, post_lb, post_ub), res


def kernel(l, u, in_lb, in_ub):
    outs, _ = _run(l, u, in_lb, in_ub)
    return outs


# revision 13
# speedup vs baseline: 2.3429x; 2.3429x over previous
"""Trainium2 Bass kernel for the ReLU bound-relaxation (CROWN-style) module.

Full inputs:  l, u: [16, 4096, 1025] f32;  in_lb, in_ub: [1024] f32
Full outputs: (l_new, u_new, post_conc_lb, post_conc_ub)

Math reformulation (validated to ~1e-7 rel err vs the jax reference):
  c = (in_lb+in_ub)/2, r = (in_ub-in_lb)/2  (r >= 0)
  S   = b + sum_e W[n,e]*c[e]          (W = eq coeffs, b = eq bias)
  t   = sum_e |W[n,e]*r[e]|            (== |W|.r since r >= 0)
  conc_lb = S_l - t_l ; max_lb = S_l + t_l ; min_ub = S_u - t_u ; conc_ub = S_u + t_u
  The ReLU relaxation is a per-row scaling:
    l_new = coef_l * l ;  u_new = coef_u * u (+ d_u added to the bias column)
  and the post-concretize pass is analytic:
    post_conc_lb = coef_l * conc_lb ;  post_conc_ub = coef_u * conc_ub + d_u

Sharding: batch axis B=16 split across 8 cores (2 batches/core); no
communication.  Per core: 8192 rows of 1025 f32 (~134 MB HBM traffic).

Implementation notes (memory-bound target, ~373 us/core HBM roofline):
  - SEG=4 consecutive rows packed per partition -> DMA partition lines are
    16.4 KB contiguous and each tile DMA moves 2.1 MB in one descriptor set
    (4 KB lines through one queue measured only ~8 GB/s/engine).
  - DMAs spread across all three issue rings: sync HWDGE (l in, post out),
    scalar HWDGE (u in), gpsimd SWDGE (l_new/u_new out).
  - Per 1025-col row-segment: DVE scalar_tensor_tensor computes
    S = sum(W_ext * [c,1]) fused (bias via the appended 1-column);
    DVE tensor_mul forms W*[r,0]; ACT activation(Abs, accum_out) reduces
    t = sum|W*r| in-place.  ACT applies coef scales in place.
  - Branchless coef math on [128, G*SEG] staging batches (~29 small DVE ops
    per group), with reciprocal-based division and comparison masks.
"""

import numpy as np

B, N, E = 16, 4096, 1025
NCORES = 8
BPC = B // NCORES          # batches per core
ROWS = BPC * N             # rows per core
P = 128                    # partitions
SEG = 4                    # consecutive rows packed per partition
TR = P * SEG               # rows per tile (512)
NT = ROWS // TR            # tiles per core (16)
G = 2                      # tiles per coef group
NG = NT // G
W = SEG * E                # tile free width (4100)
PR = ROWS // SEG           # packed-row count (2048)

_cache = {}

# DMA ring assignment: which engine issues each DMA class.
# "sync"/"scalar" = the two HWDGE rings, "gpsimd" = SWDGE ring.
RING_L_IN = "sync"
RING_U_IN = "scalar"
RING_OUT = "gpsimd"


def _build():
    from concourse import bacc, tile, mybir

    f32 = mybir.dt.float32
    Alu = mybir.AluOpType
    Act = mybir.ActivationFunctionType

    nc = bacc.Bacc("TRN2", target_bir_lowering=False, debug=False)

    # l/u viewed as [packed-row, SEG*E]; tile t = packed rows t*128..t*128+127
    l_ext = nc.dram_tensor("l", [PR, W], f32, kind="ExternalInput").ap()
    u_ext = nc.dram_tensor("u", [PR, W], f32, kind="ExternalInput").ap()
    # cbx = [c, 1.0]; rbx = [r, 0.0] broadcast to all partitions
    cb_ext = nc.dram_tensor("cbx", [P, E], f32, kind="ExternalInput").ap()
    rb_ext = nc.dram_tensor("rbx", [P, E], f32, kind="ExternalInput").ap()
    lnew_ext = nc.dram_tensor("l_new", [PR, W], f32, kind="ExternalOutput").ap()
    unew_ext = nc.dram_tensor("u_new", [PR, W], f32, kind="ExternalOutput").ap()
    plb_ext = nc.dram_tensor("post_lb", [PR, SEG], f32, kind="ExternalOutput").ap()
    pub_ext = nc.dram_tensor("post_ub", [PR, SEG], f32, kind="ExternalOutput").ap()

    GW = G * SEG  # coef stage width per group

    with tile.TileContext(nc) as tc:
        from contextlib import ExitStack

        with ExitStack() as ctx:
            consts = ctx.enter_context(tc.tile_pool(name="consts", bufs=1))
            wpool = ctx.enter_context(tc.tile_pool(name="w", bufs=G + 1))
            prodp = ctx.enter_context(tc.tile_pool(name="prod", bufs=3))
            sinkp = ctx.enter_context(tc.tile_pool(name="sink", bufs=1))
            stage = ctx.enter_context(tc.tile_pool(name="stage", bufs=2))

            cb_t = consts.tile([P, E], f32, name="cb")
            nc.sync.dma_start(cb_t[:], cb_ext[:, :])
            rb_t = consts.tile([P, E], f32, name="rb")
            nc.sync.dma_start(rb_t[:], rb_ext[:, :])
            sink = sinkp.tile([P, E], f32, name="sink")  # stt mandatory out

            seg = lambda ap, j: ap[:, j * E : (j + 1) * E]

            for grp in range(NG):
                st = lambda tag: stage.tile([P, GW], f32, name=tag, tag=tag)
                Sl, Tl, Su, Tu = st("Sl"), st("Tl"), st("Su"), st("Tu")

                lts, uts = [], []
                for gi in range(G):
                    t = grp * G + gi
                    rows = slice(t * P, (t + 1) * P)
                    lt = wpool.tile([P, W], f32, name="lt", tag="lt")
                    getattr(nc, RING_L_IN).dma_start(lt[:], l_ext[rows, :])
                    ut = wpool.tile([P, W], f32, name="ut", tag="ut")
                    getattr(nc, RING_U_IN).dma_start(ut[:], u_ext[rows, :])

                    for j in range(SEG):
                        col = gi * SEG + j
                        prl = prodp.tile([P, E], f32, name="prl", tag="prl")
                        pru = prodp.tile([P, E], f32, name="pru", tag="pru")
                        # S_l = b_l + sum(W_l * c): fused multiply+accumulate
                        nc.vector.scalar_tensor_tensor(
                            sink[:], seg(lt, j), 0.0, cb_t[:],
                            op0=Alu.bypass, op1=Alu.mult,
                            accum_out=Sl[:, col : col + 1],
                        )
                        # prl = W_l * r ; t_l = sum|prl| (ACT Abs accumulate)
                        nc.vector.tensor_mul(prl[:], seg(lt, j), rb_t[:])
                        nc.scalar.activation(
                            prl[:], prl[:], Act.Abs,
                            accum_out=Tl[:, col : col + 1])
                        nc.vector.scalar_tensor_tensor(
                            sink[:], seg(ut, j), 0.0, cb_t[:],
                            op0=Alu.bypass, op1=Alu.mult,
                            accum_out=Su[:, col : col + 1],
                        )
                        nc.vector.tensor_mul(pru[:], seg(ut, j), rb_t[:])
                        nc.scalar.activation(
                            pru[:], pru[:], Act.Abs,
                            accum_out=Tu[:, col : col + 1])
                    lts.append(lt)
                    uts.append(ut)

                # ---- branchless coef math on [P, GW] ----
                v = nc.vector
                conc_lb, max_lb, conc_ub, min_ub = (
                    st("conc_lb"), st("max_lb"), st("conc_ub"), st("min_ub"))
                v.tensor_sub(conc_lb[:], Sl[:], Tl[:])
                v.tensor_add(max_lb[:], Sl[:], Tl[:])
                v.tensor_add(conc_ub[:], Su[:], Tu[:])
                v.tensor_sub(min_ub[:], Su[:], Tu[:])

                m_ub, m_lb, sum_lu, m_ma = (
                    st("m_ub"), st("m_lb"), st("sum_lu"), st("m_ma"))
                v.tensor_scalar(m_ub[:], conc_ub[:], 0.0, None, op0=Alu.is_gt)
                v.tensor_scalar(m_lb[:], conc_lb[:], 0.0, None, op0=Alu.is_lt)
                v.tensor_add(sum_lu[:], conc_lb[:], conc_ub[:])
                v.tensor_scalar(m_ma[:], sum_lu[:], 0.0, None, op0=Alu.is_ge)

                tlc, inv_l, q, a_l = st("tlc"), st("inv_l"), st("q"), st("a_l")
                v.tensor_scalar_max(tlc[:], Tl[:], 1e-35)
                v.reciprocal(inv_l[:], tlc[:])
                v.scalar_tensor_tensor(
                    q[:], max_lb[:], 0.5, inv_l[:], op0=Alu.mult, op1=Alu.mult)
                v.tensor_scalar_max(a_l[:], q[:], 0.0)

                cu0, x, y, coef_l, post_l = (
                    st("cu0"), st("x"), st("y"), st("coef_l"), st("post_l"))
                v.tensor_mul(cu0[:], m_ma[:], a_l[:])
                v.tensor_mul(x[:], m_lb[:], cu0[:])
                v.scalar_tensor_tensor(
                    y[:], x[:], 1.0, m_lb[:], op0=Alu.add, op1=Alu.subtract)
                v.tensor_mul(coef_l[:], m_ub[:], y[:])
                v.tensor_mul(post_l[:], coef_l[:], conc_lb[:])

                tuc, inv_u, a_u = st("tuc"), st("inv_u"), st("a_u")
                v.tensor_scalar_max(tuc[:], Tu[:], 1e-35)
                v.reciprocal(inv_u[:], tuc[:])
                v.scalar_tensor_tensor(
                    a_u[:], conc_ub[:], 0.5, inv_u[:], op0=Alu.mult, op1=Alu.mult)

                m_zc, unst, zc, xu, yu = (
                    st("m_zc"), st("unst"), st("zc"), st("xu"), st("yu"))
                v.tensor_scalar(m_zc[:], min_ub[:], 0.0, None, op0=Alu.is_le)
                v.tensor_mul(unst[:], m_lb[:], m_ub[:])
                v.tensor_mul(zc[:], unst[:], m_zc[:])
                v.tensor_mul(xu[:], zc[:], a_u[:])
                v.scalar_tensor_tensor(
                    yu[:], xu[:], 1.0, zc[:], op0=Alu.add, op1=Alu.subtract)

                coef_u, du, pu0, post_u = (
                    st("coef_u"), st("du"), st("pu0"), st("post_u"))
                v.tensor_mul(coef_u[:], m_ub[:], yu[:])
                v.scalar_tensor_tensor(
                    du[:], min_ub[:], -1.0, xu[:], op0=Alu.mult, op1=Alu.mult)
                v.tensor_mul(pu0[:], coef_u[:], conc_ub[:])
                v.tensor_add(post_u[:], pu0[:], du[:])

                # ---- apply scales in-place and DMA out ----
                for gi in range(G):
                    t = grp * G + gi
                    rows = slice(t * P, (t + 1) * P)
                    lt, ut = lts[gi], uts[gi]
                    for j in range(SEG):
                        col = gi * SEG + j
                        nc.scalar.activation(
                            seg(lt, j), seg(lt, j), Act.Copy, bias=0.0,
                            scale=coef_l[:, col : col + 1])
                        nc.scalar.activation(
                            ut[:, j * E : j * E + E - 1],
                            ut[:, j * E : j * E + E - 1],
                            Act.Copy, bias=0.0,
                            scale=coef_u[:, col : col + 1])
                        nc.scalar.activation(
                            ut[:, j * E + E - 1 : j * E + E],
                            ut[:, j * E + E - 1 : j * E + E],
                            Act.Identity,
                            bias=du[:, col : col + 1],
                            scale=coef_u[:, col : col + 1])
                    getattr(nc, RING_OUT).dma_start(lnew_ext[rows, :], lt[:])
                    getattr(nc, RING_OUT).dma_start(unew_ext[rows, :], ut[:])
                    nc.sync.dma_start(
                        plb_ext[rows, :], post_l[:, gi * SEG : (gi + 1) * SEG])
                    nc.sync.dma_start(
                        pub_ext[rows, :], post_u[:, gi * SEG : (gi + 1) * SEG])

    nc.compile()
    return nc


def _get_nc():
    if "nc" not in _cache:
        _cache["nc"] = _build()
    return _cache["nc"]


def _run(l, u, in_lb, in_ub, trace=False, runner=None):
    from concourse.bass_utils import run_bass_kernel_spmd

    nc = _get_nc()

    l = np.ascontiguousarray(l, dtype=np.float32)
    u = np.ascontiguousarray(u, dtype=np.float32)
    in_lb = np.asarray(in_lb, dtype=np.float32)
    in_ub = np.asarray(in_ub, dtype=np.float32)

    c = (in_lb + in_ub) * np.float32(0.5)
    r = (in_ub - in_lb) * np.float32(0.5)
    cbx = np.concatenate([c, np.ones(1, np.float32)])
    rbx = np.concatenate([r, np.zeros(1, np.float32)])
    cb = np.ascontiguousarray(np.broadcast_to(cbx, (P, E)), dtype=np.float32)
    rb = np.ascontiguousarray(np.broadcast_to(rbx, (P, E)), dtype=np.float32)

    in_maps = []
    for i in range(NCORES):
        sl = slice(i * BPC, (i + 1) * BPC)
        in_maps.append({
            "l": l[sl].reshape(PR, W),
            "u": u[sl].reshape(PR, W),
            "cbx": cb,
            "rbx": rb,
        })

    if runner is not None:
        res = runner(nc, in_maps)
    else:
        res = run_bass_kernel_spmd(nc, in_maps, core_ids=list(range(NCORES)),
                                   trace=trace)

    l_new = np.empty((B, N, E), dtype=np.float32)
    u_new = np.empty((B, N, E), dtype=np.float32)
    post_lb = np.empty((B, N), dtype=np.float32)
    post_ub = np.empty((B, N), dtype=np.float32)
    for i in range(NCORES):
        sl = slice(i * BPC, (i + 1) * BPC)
        out = res.results[i]
        l_new[sl] = np.asarray(out["l_new"]).reshape(BPC, N, E)
        u_new[sl] = np.asarray(out["u_new"]).reshape(BPC, N, E)
        post_lb[sl] = np.asarray(out["post_lb"]).reshape(BPC, N)
        post_ub[sl] = np.asarray(out["post_ub"]).reshape(BPC, N)

    return (l_new, u_new, post_lb, post_ub), res


def kernel(l, u, in_lb, in_ub):
    outs, _ = _run(l, u, in_lb, in_ub)
    return outs


# revision 17
# speedup vs baseline: 2.6362x; 1.1252x over previous
"""Trainium2 Bass kernel for the ReLU bound-relaxation (CROWN-style) module.

Full inputs:  l, u: [16, 4096, 1025] f32;  in_lb, in_ub: [1024] f32
Full outputs: (l_new, u_new, post_conc_lb, post_conc_ub)

Math reformulation (validated to ~1e-7 rel err vs the jax reference):
  c = (in_lb+in_ub)/2, r = (in_ub-in_lb)/2  (r >= 0)
  S   = b + sum_e W[n,e]*c[e]          (W = eq coeffs, b = eq bias)
  t   = sum_e |W[n,e]*r[e]|            (== |W|.r since r >= 0)
  conc_lb = S_l - t_l ; max_lb = S_l + t_l ; min_ub = S_u - t_u ; conc_ub = S_u + t_u
  The ReLU relaxation is a per-row scaling:
    l_new = coef_l * l ;  u_new = coef_u * u (+ d_u added to the bias column)
  and the post-concretize pass is analytic:
    post_conc_lb = coef_l * conc_lb ;  post_conc_ub = coef_u * conc_ub + d_u

Sharding: batch axis B=16 split across 8 cores (2 batches/core); no
communication.  Per core: 8192 rows of 1025 f32 (~134 MB HBM traffic).

Implementation notes (memory-bound target, ~373 us/core HBM roofline):
  - SEG=4 consecutive rows packed per partition -> DMA partition lines are
    16.4 KB contiguous and each tile DMA moves 2.1 MB in one descriptor set
    (4 KB lines through one queue measured only ~8 GB/s/engine).
  - DMAs spread across all three issue rings: sync HWDGE (l in, post out),
    scalar HWDGE (u in), gpsimd SWDGE (l_new/u_new out).
  - Per 1025-col row-segment: DVE scalar_tensor_tensor computes
    S = sum(W_ext * [c,1]) fused (bias via the appended 1-column);
    DVE tensor_mul forms W*[r,0]; ACT activation(Abs, accum_out) reduces
    t = sum|W*r| in-place.  ACT applies coef scales in place.
  - Branchless coef math on [128, G*SEG] staging batches (~29 small DVE ops
    per group), with reciprocal-based division and comparison masks.
"""

import numpy as np

B, N, E = 16, 4096, 1025
NCORES = 8
BPC = B // NCORES          # batches per core
ROWS = BPC * N             # rows per core
P = 128                    # partitions
SEG = 4                    # consecutive rows packed per partition
TR = P * SEG               # rows per tile (512)
NT = ROWS // TR            # tiles per core (16)
G = 2                      # tiles per coef group
NG = NT // G
W = SEG * E                # tile free width (4100)
PR = ROWS // SEG           # packed-row count (2048)

_cache = {}

# DMA ring assignment: which engine issues each DMA class.
# "sync"/"scalar" = the two HWDGE rings, "gpsimd" = SWDGE ring.
RING_L_IN = "sync"
RING_U_IN = "scalar"
RING_OUT = "gpsimd"


def _build():
    from concourse import bacc, tile, mybir

    f32 = mybir.dt.float32
    Alu = mybir.AluOpType
    Act = mybir.ActivationFunctionType

    nc = bacc.Bacc("TRN2", target_bir_lowering=False, debug=False)

    # l/u viewed as [packed-row, SEG*E]; tile t = packed rows t*128..t*128+127
    l_ext = nc.dram_tensor("l", [PR, W], f32, kind="ExternalInput").ap()
    u_ext = nc.dram_tensor("u", [PR, W], f32, kind="ExternalInput").ap()
    # cbx = [c, 1.0]; rbx = [r, 0.0] broadcast to all partitions
    cb_ext = nc.dram_tensor("cbx", [P, E], f32, kind="ExternalInput").ap()
    rb_ext = nc.dram_tensor("rbx", [P, E], f32, kind="ExternalInput").ap()
    lnew_ext = nc.dram_tensor("l_new", [PR, W], f32, kind="ExternalOutput").ap()
    unew_ext = nc.dram_tensor("u_new", [PR, W], f32, kind="ExternalOutput").ap()
    plb_ext = nc.dram_tensor("post_lb", [PR, SEG], f32, kind="ExternalOutput").ap()
    pub_ext = nc.dram_tensor("post_ub", [PR, SEG], f32, kind="ExternalOutput").ap()

    GW = G * SEG  # coef stage width per group

    with tile.TileContext(nc) as tc:
        from contextlib import ExitStack

        with ExitStack() as ctx:
            consts = ctx.enter_context(tc.tile_pool(name="consts", bufs=1))
            wpool = ctx.enter_context(tc.tile_pool(name="w", bufs=2 * G))
            prodp = ctx.enter_context(tc.tile_pool(name="prod", bufs=3))
            sinkp = ctx.enter_context(tc.tile_pool(name="sink", bufs=1))
            stage = ctx.enter_context(tc.tile_pool(name="stage", bufs=2))

            cb_t = consts.tile([P, E], f32, name="cb")
            nc.sync.dma_start(cb_t[:], cb_ext[:, :])
            rb_t = consts.tile([P, E], f32, name="rb")
            nc.sync.dma_start(rb_t[:], rb_ext[:, :])
            sink = sinkp.tile([P, E], f32, name="sink")  # stt mandatory out

            seg = lambda ap, j: ap[:, j * E : (j + 1) * E]

            # emit the scale+store steps for tile gi, segment j of a
            # finished group (interleaved into the next group's dot loop so
            # ACT never head-of-line blocks on the coef chain)
            def emit_scale_step(pv, gi, j):
                (p_lts, p_uts, p_coef_l, p_coef_u, p_du, p_post_l,
                 p_post_u, p_grp) = pv
                lt, ut = p_lts[gi], p_uts[gi]
                col = gi * SEG + j
                nc.scalar.activation(
                    seg(lt, j), seg(lt, j), Act.Copy, bias=0.0,
                    scale=p_coef_l[:, col : col + 1])
                nc.scalar.activation(
                    ut[:, j * E : j * E + E - 1],
                    ut[:, j * E : j * E + E - 1],
                    Act.Copy, bias=0.0,
                    scale=p_coef_u[:, col : col + 1])
                nc.scalar.activation(
                    ut[:, j * E + E - 1 : j * E + E],
                    ut[:, j * E + E - 1 : j * E + E],
                    Act.Identity,
                    bias=p_du[:, col : col + 1],
                    scale=p_coef_u[:, col : col + 1])
                if j == SEG - 1:
                    t = p_grp * G + gi
                    rows = slice(t * P, (t + 1) * P)
                    getattr(nc, RING_OUT).dma_start(lnew_ext[rows, :], lt[:])
                    getattr(nc, RING_OUT).dma_start(unew_ext[rows, :], ut[:])
                    nc.sync.dma_start(
                        plb_ext[rows, :],
                        p_post_l[:, gi * SEG : (gi + 1) * SEG])
                    nc.sync.dma_start(
                        pub_ext[rows, :],
                        p_post_u[:, gi * SEG : (gi + 1) * SEG])

            prev = None
            for grp in range(NG):
                st = lambda tag: stage.tile([P, GW], f32, name=tag, tag=tag)
                Sl, Tl, Su, Tu = st("Sl"), st("Tl"), st("Su"), st("Tu")

                lts, uts = [], []
                for gi in range(G):
                    t = grp * G + gi
                    rows = slice(t * P, (t + 1) * P)
                    lt = wpool.tile([P, W], f32, name="lt", tag="lt")
                    getattr(nc, RING_L_IN).dma_start(lt[:], l_ext[rows, :])
                    ut = wpool.tile([P, W], f32, name="ut", tag="ut")
                    getattr(nc, RING_U_IN).dma_start(ut[:], u_ext[rows, :])

                    for j in range(SEG):
                        col = gi * SEG + j
                        if prev is not None:
                            emit_scale_step(prev, gi, j)
                        prl = prodp.tile([P, E], f32, name="prl", tag="prl")
                        pru = prodp.tile([P, E], f32, name="pru", tag="pru")
                        # S_l = b_l + sum(W_l * c): fused multiply+accumulate
                        nc.vector.scalar_tensor_tensor(
                            sink[:], seg(lt, j), 0.0, cb_t[:],
                            op0=Alu.bypass, op1=Alu.mult,
                            accum_out=Sl[:, col : col + 1],
                        )
                        # prl = W_l * r ; t_l = sum|prl| (ACT Abs accumulate)
                        nc.vector.tensor_mul(prl[:], seg(lt, j), rb_t[:])
                        nc.scalar.activation(
                            prl[:], prl[:], Act.Abs,
                            accum_out=Tl[:, col : col + 1])
                        nc.vector.scalar_tensor_tensor(
                            sink[:], seg(ut, j), 0.0, cb_t[:],
                            op0=Alu.bypass, op1=Alu.mult,
                            accum_out=Su[:, col : col + 1],
                        )
                        nc.vector.tensor_mul(pru[:], seg(ut, j), rb_t[:])
                        nc.scalar.activation(
                            pru[:], pru[:], Act.Abs,
                            accum_out=Tu[:, col : col + 1])
                    lts.append(lt)
                    uts.append(ut)

                # ---- branchless coef math on [P, GW] ----
                v = nc.vector
                conc_lb, max_lb, conc_ub, min_ub = (
                    st("conc_lb"), st("max_lb"), st("conc_ub"), st("min_ub"))
                v.tensor_sub(conc_lb[:], Sl[:], Tl[:])
                v.tensor_add(max_lb[:], Sl[:], Tl[:])
                v.tensor_add(conc_ub[:], Su[:], Tu[:])
                v.tensor_sub(min_ub[:], Su[:], Tu[:])

                m_ub, m_lb, sum_lu, m_ma = (
                    st("m_ub"), st("m_lb"), st("sum_lu"), st("m_ma"))
                v.tensor_scalar(m_ub[:], conc_ub[:], 0.0, None, op0=Alu.is_gt)
                v.tensor_scalar(m_lb[:], conc_lb[:], 0.0, None, op0=Alu.is_lt)
                v.tensor_add(sum_lu[:], conc_lb[:], conc_ub[:])
                v.tensor_scalar(m_ma[:], sum_lu[:], 0.0, None, op0=Alu.is_ge)

                tlc, inv_l, q, a_l = st("tlc"), st("inv_l"), st("q"), st("a_l")
                v.tensor_scalar_max(tlc[:], Tl[:], 1e-35)
                v.reciprocal(inv_l[:], tlc[:])
                v.scalar_tensor_tensor(
                    q[:], max_lb[:], 0.5, inv_l[:], op0=Alu.mult, op1=Alu.mult)
                v.tensor_scalar_max(a_l[:], q[:], 0.0)

                cu0, x, y, coef_l, post_l = (
                    st("cu0"), st("x"), st("y"), st("coef_l"), st("post_l"))
                v.tensor_mul(cu0[:], m_ma[:], a_l[:])
                v.tensor_mul(x[:], m_lb[:], cu0[:])
                v.scalar_tensor_tensor(
                    y[:], x[:], 1.0, m_lb[:], op0=Alu.add, op1=Alu.subtract)
                v.tensor_mul(coef_l[:], m_ub[:], y[:])
                v.tensor_mul(post_l[:], coef_l[:], conc_lb[:])

                tuc, inv_u, a_u = st("tuc"), st("inv_u"), st("a_u")
                v.tensor_scalar_max(tuc[:], Tu[:], 1e-35)
                v.reciprocal(inv_u[:], tuc[:])
                v.scalar_tensor_tensor(
                    a_u[:], conc_ub[:], 0.5, inv_u[:], op0=Alu.mult, op1=Alu.mult)

                m_zc, unst, zc, xu, yu = (
                    st("m_zc"), st("unst"), st("zc"), st("xu"), st("yu"))
                v.tensor_scalar(m_zc[:], min_ub[:], 0.0, None, op0=Alu.is_le)
                v.tensor_mul(unst[:], m_lb[:], m_ub[:])
                v.tensor_mul(zc[:], unst[:], m_zc[:])
                v.tensor_mul(xu[:], zc[:], a_u[:])
                v.scalar_tensor_tensor(
                    yu[:], xu[:], 1.0, zc[:], op0=Alu.add, op1=Alu.subtract)

                coef_u, du, pu0, post_u = (
                    st("coef_u"), st("du"), st("pu0"), st("post_u"))
                v.tensor_mul(coef_u[:], m_ub[:], yu[:])
                v.scalar_tensor_tensor(
                    du[:], min_ub[:], -1.0, xu[:], op0=Alu.mult, op1=Alu.mult)
                v.tensor_mul(pu0[:], coef_u[:], conc_ub[:])
                v.tensor_add(post_u[:], pu0[:], du[:])

                prev = (lts, uts, coef_l, coef_u, du, post_l, post_u, grp)

            # drain: scales + stores for the final group
            for gi in range(G):
                for j in range(SEG):
                    emit_scale_step(prev, gi, j)

    nc.compile()
    return nc


def _get_nc():
    if "nc" not in _cache:
        _cache["nc"] = _build()
    return _cache["nc"]


def _run(l, u, in_lb, in_ub, trace=False, runner=None):
    from concourse.bass_utils import run_bass_kernel_spmd

    nc = _get_nc()

    l = np.ascontiguousarray(l, dtype=np.float32)
    u = np.ascontiguousarray(u, dtype=np.float32)
    in_lb = np.asarray(in_lb, dtype=np.float32)
    in_ub = np.asarray(in_ub, dtype=np.float32)

    c = (in_lb + in_ub) * np.float32(0.5)
    r = (in_ub - in_lb) * np.float32(0.5)
    cbx = np.concatenate([c, np.ones(1, np.float32)])
    rbx = np.concatenate([r, np.zeros(1, np.float32)])
    cb = np.ascontiguousarray(np.broadcast_to(cbx, (P, E)), dtype=np.float32)
    rb = np.ascontiguousarray(np.broadcast_to(rbx, (P, E)), dtype=np.float32)

    in_maps = []
    for i in range(NCORES):
        sl = slice(i * BPC, (i + 1) * BPC)
        in_maps.append({
            "l": l[sl].reshape(PR, W),
            "u": u[sl].reshape(PR, W),
            "cbx": cb,
            "rbx": rb,
        })

    if runner is not None:
        res = runner(nc, in_maps)
    else:
        res = run_bass_kernel_spmd(nc, in_maps, core_ids=list(range(NCORES)),
                                   trace=trace)

    l_new = np.empty((B, N, E), dtype=np.float32)
    u_new = np.empty((B, N, E), dtype=np.float32)
    post_lb = np.empty((B, N), dtype=np.float32)
    post_ub = np.empty((B, N), dtype=np.float32)
    for i in range(NCORES):
        sl = slice(i * BPC, (i + 1) * BPC)
        out = res.results[i]
        l_new[sl] = np.asarray(out["l_new"]).reshape(BPC, N, E)
        u_new[sl] = np.asarray(out["u_new"]).reshape(BPC, N, E)
        post_lb[sl] = np.asarray(out["post_lb"]).reshape(BPC, N)
        post_ub[sl] = np.asarray(out["post_ub"]).reshape(BPC, N)

    return (l_new, u_new, post_lb, post_ub), res


def kernel(l, u, in_lb, in_ub):
    outs, _ = _run(l, u, in_lb, in_ub)
    return outs


# revision 18
# speedup vs baseline: 2.9076x; 1.1029x over previous
"""Trainium2 Bass kernel for the ReLU bound-relaxation (CROWN-style) module.

Full inputs:  l, u: [16, 4096, 1025] f32;  in_lb, in_ub: [1024] f32
Full outputs: (l_new, u_new, post_conc_lb, post_conc_ub)

Math reformulation (validated to ~1e-7 rel err vs the jax reference):
  c = (in_lb+in_ub)/2, r = (in_ub-in_lb)/2  (r >= 0)
  S   = b + sum_e W[n,e]*c[e]          (W = eq coeffs, b = eq bias)
  t   = sum_e |W[n,e]*r[e]|            (== |W|.r since r >= 0)
  conc_lb = S_l - t_l ; max_lb = S_l + t_l ; min_ub = S_u - t_u ; conc_ub = S_u + t_u
  The ReLU relaxation is a per-row scaling:
    l_new = coef_l * l ;  u_new = coef_u * u (+ d_u added to the bias column)
  and the post-concretize pass is analytic:
    post_conc_lb = coef_l * conc_lb ;  post_conc_ub = coef_u * conc_ub + d_u

Sharding: batch axis B=16 split across 8 cores (2 batches/core); no
communication.  Per core: 8192 rows of 1025 f32 (~134 MB HBM traffic).

Implementation notes (memory-bound target, ~373 us/core HBM roofline):
  - SEG=4 consecutive rows packed per partition -> DMA partition lines are
    16.4 KB contiguous and each tile DMA moves 2.1 MB in one descriptor set
    (4 KB lines through one queue measured only ~8 GB/s/engine).
  - DMAs spread across all three issue rings: sync HWDGE (l in, post out),
    scalar HWDGE (u in), gpsimd SWDGE (l_new/u_new out).
  - Per 1025-col row-segment: DVE scalar_tensor_tensor computes
    S = sum(W_ext * [c,1]) fused (bias via the appended 1-column);
    DVE tensor_mul forms W*[r,0]; ACT activation(Abs, accum_out) reduces
    t = sum|W*r| in-place.  ACT applies coef scales in place.
  - Branchless coef math on [128, G*SEG] staging batches (~29 small DVE ops
    per group), with reciprocal-based division and comparison masks.
"""

import numpy as np

B, N, E = 16, 4096, 1025
NCORES = 8
BPC = B // NCORES          # batches per core
ROWS = BPC * N             # rows per core
P = 128                    # partitions
SEG = 4                    # consecutive rows packed per partition
TR = P * SEG               # rows per tile (512)
NT = ROWS // TR            # tiles per core (16)
G = 2                      # tiles per coef group
NG = NT // G
W = SEG * E                # tile free width (4100)
PR = ROWS // SEG           # packed-row count (2048)

_cache = {}

# DMA ring assignment: which engine issues each DMA class.
# "sync"/"scalar" = the two HWDGE rings, "gpsimd" = SWDGE ring.
RING_L_IN = "sync"
RING_U_IN = "scalar"
RING_OUT = "gpsimd"


def _build():
    from concourse import bacc, tile, mybir

    f32 = mybir.dt.float32
    Alu = mybir.AluOpType
    Act = mybir.ActivationFunctionType

    nc = bacc.Bacc("TRN2", target_bir_lowering=False, debug=False)

    # l/u viewed as [packed-row, SEG*E]; tile t = packed rows t*128..t*128+127
    l_ext = nc.dram_tensor("l", [PR, W], f32, kind="ExternalInput").ap()
    u_ext = nc.dram_tensor("u", [PR, W], f32, kind="ExternalInput").ap()
    # cbx = [c, 1.0]; rbx = [r, 0.0] broadcast to all partitions
    cb_ext = nc.dram_tensor("cbx", [P, E], f32, kind="ExternalInput").ap()
    rb_ext = nc.dram_tensor("rbx", [P, E], f32, kind="ExternalInput").ap()
    lnew_ext = nc.dram_tensor("l_new", [PR, W], f32, kind="ExternalOutput").ap()
    unew_ext = nc.dram_tensor("u_new", [PR, W], f32, kind="ExternalOutput").ap()
    plb_ext = nc.dram_tensor("post_lb", [PR, SEG], f32, kind="ExternalOutput").ap()
    pub_ext = nc.dram_tensor("post_ub", [PR, SEG], f32, kind="ExternalOutput").ap()

    GW = G * SEG  # coef stage width per group

    with tile.TileContext(nc) as tc:
        from contextlib import ExitStack

        with ExitStack() as ctx:
            consts = ctx.enter_context(tc.tile_pool(name="consts", bufs=1))
            wpool = ctx.enter_context(tc.tile_pool(name="w", bufs=2 * G + 1))
            prodp = ctx.enter_context(tc.tile_pool(name="prod", bufs=3))
            sinkp = ctx.enter_context(tc.tile_pool(name="sink", bufs=1))
            stage = ctx.enter_context(tc.tile_pool(name="stage", bufs=2))

            cb_t = consts.tile([P, E], f32, name="cb")
            nc.sync.dma_start(cb_t[:], cb_ext[:, :])
            rb_t = consts.tile([P, E], f32, name="rb")
            nc.sync.dma_start(rb_t[:], rb_ext[:, :])
            sink = sinkp.tile([P, E], f32, name="sink")  # stt mandatory out

            seg = lambda ap, j: ap[:, j * E : (j + 1) * E]

            # emit the scale+store steps for tile gi, segment j of a
            # finished group (interleaved into the next group's dot loop so
            # ACT never head-of-line blocks on the coef chain)
            def emit_scale_step(pv, gi, j):
                (p_lts, p_uts, p_coef_l, p_coef_u, p_du, p_post_l,
                 p_post_u, p_grp) = pv
                lt, ut = p_lts[gi], p_uts[gi]
                col = gi * SEG + j
                nc.scalar.activation(
                    seg(lt, j), seg(lt, j), Act.Copy, bias=0.0,
                    scale=p_coef_l[:, col : col + 1])
                nc.scalar.activation(
                    ut[:, j * E : j * E + E - 1],
                    ut[:, j * E : j * E + E - 1],
                    Act.Copy, bias=0.0,
                    scale=p_coef_u[:, col : col + 1])
                nc.scalar.activation(
                    ut[:, j * E + E - 1 : j * E + E],
                    ut[:, j * E + E - 1 : j * E + E],
                    Act.Identity,
                    bias=p_du[:, col : col + 1],
                    scale=p_coef_u[:, col : col + 1])
                if j == SEG - 1:
                    t = p_grp * G + gi
                    rows = slice(t * P, (t + 1) * P)
                    getattr(nc, RING_OUT).dma_start(lnew_ext[rows, :], lt[:])
                    getattr(nc, RING_OUT).dma_start(unew_ext[rows, :], ut[:])
                    nc.sync.dma_start(
                        plb_ext[rows, :],
                        p_post_l[:, gi * SEG : (gi + 1) * SEG])
                    nc.sync.dma_start(
                        pub_ext[rows, :],
                        p_post_u[:, gi * SEG : (gi + 1) * SEG])

            prev = None
            for grp in range(NG):
                st = lambda tag: stage.tile([P, GW], f32, name=tag, tag=tag)
                Sl, Tl, Su, Tu = st("Sl"), st("Tl"), st("Su"), st("Tu")

                lts, uts = [], []
                for gi in range(G):
                    t = grp * G + gi
                    rows = slice(t * P, (t + 1) * P)
                    lt = wpool.tile([P, W], f32, name="lt", tag="lt")
                    getattr(nc, RING_L_IN).dma_start(lt[:], l_ext[rows, :])
                    ut = wpool.tile([P, W], f32, name="ut", tag="ut")
                    getattr(nc, RING_U_IN).dma_start(ut[:], u_ext[rows, :])

                    for j in range(SEG):
                        col = gi * SEG + j
                        if prev is not None:
                            emit_scale_step(prev, gi, j)
                        prl = prodp.tile([P, E], f32, name="prl", tag="prl")
                        pru = prodp.tile([P, E], f32, name="pru", tag="pru")
                        # S_l = b_l + sum(W_l * c): fused multiply+accumulate
                        nc.vector.scalar_tensor_tensor(
                            sink[:], seg(lt, j), 0.0, cb_t[:],
                            op0=Alu.bypass, op1=Alu.mult,
                            accum_out=Sl[:, col : col + 1],
                        )
                        # prl = W_l * r ; t_l = sum|prl| (ACT Abs accumulate)
                        nc.vector.tensor_mul(prl[:], seg(lt, j), rb_t[:])
                        nc.scalar.activation(
                            prl[:], prl[:], Act.Abs,
                            accum_out=Tl[:, col : col + 1])
                        nc.vector.scalar_tensor_tensor(
                            sink[:], seg(ut, j), 0.0, cb_t[:],
                            op0=Alu.bypass, op1=Alu.mult,
                            accum_out=Su[:, col : col + 1],
                        )
                        nc.vector.tensor_mul(pru[:], seg(ut, j), rb_t[:])
                        nc.scalar.activation(
                            pru[:], pru[:], Act.Abs,
                            accum_out=Tu[:, col : col + 1])
                    lts.append(lt)
                    uts.append(ut)

                # ---- branchless coef math on [P, GW] ----
                v = nc.vector
                conc_lb, max_lb, conc_ub, min_ub = (
                    st("conc_lb"), st("max_lb"), st("conc_ub"), st("min_ub"))
                v.tensor_sub(conc_lb[:], Sl[:], Tl[:])
                v.tensor_add(max_lb[:], Sl[:], Tl[:])
                v.tensor_add(conc_ub[:], Su[:], Tu[:])
                v.tensor_sub(min_ub[:], Su[:], Tu[:])

                m_ub, m_lb, sum_lu, m_ma = (
                    st("m_ub"), st("m_lb"), st("sum_lu"), st("m_ma"))
                v.tensor_scalar(m_ub[:], conc_ub[:], 0.0, None, op0=Alu.is_gt)
                v.tensor_scalar(m_lb[:], conc_lb[:], 0.0, None, op0=Alu.is_lt)
                v.tensor_add(sum_lu[:], conc_lb[:], conc_ub[:])
                v.tensor_scalar(m_ma[:], sum_lu[:], 0.0, None, op0=Alu.is_ge)

                tlc, inv_l, q, a_l = st("tlc"), st("inv_l"), st("q"), st("a_l")
                v.tensor_scalar_max(tlc[:], Tl[:], 1e-35)
                v.reciprocal(inv_l[:], tlc[:])
                v.scalar_tensor_tensor(
                    q[:], max_lb[:], 0.5, inv_l[:], op0=Alu.mult, op1=Alu.mult)
                v.tensor_scalar_max(a_l[:], q[:], 0.0)

                cu0, x, y, coef_l, post_l = (
                    st("cu0"), st("x"), st("y"), st("coef_l"), st("post_l"))
                v.tensor_mul(cu0[:], m_ma[:], a_l[:])
                v.tensor_mul(x[:], m_lb[:], cu0[:])
                v.scalar_tensor_tensor(
                    y[:], x[:], 1.0, m_lb[:], op0=Alu.add, op1=Alu.subtract)
                v.tensor_mul(coef_l[:], m_ub[:], y[:])
                v.tensor_mul(post_l[:], coef_l[:], conc_lb[:])

                tuc, inv_u, a_u = st("tuc"), st("inv_u"), st("a_u")
                v.tensor_scalar_max(tuc[:], Tu[:], 1e-35)
                v.reciprocal(inv_u[:], tuc[:])
                v.scalar_tensor_tensor(
                    a_u[:], conc_ub[:], 0.5, inv_u[:], op0=Alu.mult, op1=Alu.mult)

                m_zc, unst, zc, xu, yu = (
                    st("m_zc"), st("unst"), st("zc"), st("xu"), st("yu"))
                v.tensor_scalar(m_zc[:], min_ub[:], 0.0, None, op0=Alu.is_le)
                v.tensor_mul(unst[:], m_lb[:], m_ub[:])
                v.tensor_mul(zc[:], unst[:], m_zc[:])
                v.tensor_mul(xu[:], zc[:], a_u[:])
                v.scalar_tensor_tensor(
                    yu[:], xu[:], 1.0, zc[:], op0=Alu.add, op1=Alu.subtract)

                coef_u, du, pu0, post_u = (
                    st("coef_u"), st("du"), st("pu0"), st("post_u"))
                v.tensor_mul(coef_u[:], m_ub[:], yu[:])
                v.scalar_tensor_tensor(
                    du[:], min_ub[:], -1.0, xu[:], op0=Alu.mult, op1=Alu.mult)
                v.tensor_mul(pu0[:], coef_u[:], conc_ub[:])
                v.tensor_add(post_u[:], pu0[:], du[:])

                prev = (lts, uts, coef_l, coef_u, du, post_l, post_u, grp)

            # drain: scales + stores for the final group
            for gi in range(G):
                for j in range(SEG):
                    emit_scale_step(prev, gi, j)

    nc.compile()
    return nc


def _get_nc():
    if "nc" not in _cache:
        _cache["nc"] = _build()
    return _cache["nc"]


def _run(l, u, in_lb, in_ub, trace=False, runner=None):
    from concourse.bass_utils import run_bass_kernel_spmd

    nc = _get_nc()

    l = np.ascontiguousarray(l, dtype=np.float32)
    u = np.ascontiguousarray(u, dtype=np.float32)
    in_lb = np.asarray(in_lb, dtype=np.float32)
    in_ub = np.asarray(in_ub, dtype=np.float32)

    c = (in_lb + in_ub) * np.float32(0.5)
    r = (in_ub - in_lb) * np.float32(0.5)
    cbx = np.concatenate([c, np.ones(1, np.float32)])
    rbx = np.concatenate([r, np.zeros(1, np.float32)])
    cb = np.ascontiguousarray(np.broadcast_to(cbx, (P, E)), dtype=np.float32)
    rb = np.ascontiguousarray(np.broadcast_to(rbx, (P, E)), dtype=np.float32)

    in_maps = []
    for i in range(NCORES):
        sl = slice(i * BPC, (i + 1) * BPC)
        in_maps.append({
            "l": l[sl].reshape(PR, W),
            "u": u[sl].reshape(PR, W),
            "cbx": cb,
            "rbx": rb,
        })

    if runner is not None:
        res = runner(nc, in_maps)
    else:
        res = run_bass_kernel_spmd(nc, in_maps, core_ids=list(range(NCORES)),
                                   trace=trace)

    l_new = np.empty((B, N, E), dtype=np.float32)
    u_new = np.empty((B, N, E), dtype=np.float32)
    post_lb = np.empty((B, N), dtype=np.float32)
    post_ub = np.empty((B, N), dtype=np.float32)
    for i in range(NCORES):
        sl = slice(i * BPC, (i + 1) * BPC)
        out = res.results[i]
        l_new[sl] = np.asarray(out["l_new"]).reshape(BPC, N, E)
        u_new[sl] = np.asarray(out["u_new"]).reshape(BPC, N, E)
        post_lb[sl] = np.asarray(out["post_lb"]).reshape(BPC, N)
        post_ub[sl] = np.asarray(out["post_ub"]).reshape(BPC, N)

    return (l_new, u_new, post_lb, post_ub), res


def kernel(l, u, in_lb, in_ub):
    outs, _ = _run(l, u, in_lb, in_ub)
    return outs
